# revision 23
# baseline (speedup 1.0000x reference)
"""Trainium2 Bass kernel for the ConvE-style MoE-routing block.

Computes, for each batch row b:
    X = [e1|e2] @ rel_emb.T            # [B, NR] gating logits
    S, idx = top_k(sigmoid(X), 16)
    R1 = relu(rel_emb @ W_fcs.T + b)   # [NR, D]
    out = sum_k S_k * R1[idx_k] / sum_k S_k

Reformulated gather-free: zap the top-16 logits per row with two
(max8 + match_replace) rounds, then M = sigmoid(X) - sigmoid(X_zapped)
is exactly the top-16 sigmoid weights (0 elsewhere), so
    out = (M @ R1) / rowsum(M)
runs on the tensor engine as a dense matmul.

Precision: fp32 matmul costs 4 PE cycles/row and f32r (1 cycle/row)
truncates inputs to ~bf16, which flips ~25% of rows' top-16 sets. The
gating here instead uses a 3-term fp16 split at full PE rate:
    X*SA*SB = a_hi@b_hi + a_lo@b_hi + a_hi@b_lo
with a = stacked*SA, b = rel*SB pre-split hi/lo into fp16 host-side.
The scales keep every split value in fp16 normal range (subnormal-FTZ
safe); residual error ~2e-7 abs vs top-16 gap scale ~1.5e-2, giving 0
selection flips on N(0,1)-type data (validated offline vs fp64).
Sigmoids fold the 1/(SA*SB) rescale into the ACT input scale. R1 and
the combine matmul are fp16 (value-grade).

Layouts: the PE contracts along partitions, so all contraction
operands are marshalled host-side in numpy (pure input marshalling, no
FLOPs) into SBUF-native [128, free] layouts and DMA'd with one
descriptor each; the kernel spends no engine time on transposes except
M^T (data-dependent, via DMA xbar).

Data-parallel over batch across 8 cores; rel/W replicated; R1 computed
fully locally on every core straight from the already-loaded scaled
rel_hi chunks (an AllGather of a sharded R1 measured ~90us of
collective latency and stalled the in-order PE queue — recomputing is
144 matmuls ~= 38us and makes the cores fully independent). Combines
trail gating by PEND tiles so the PE never waits on the serial DVE
top-k chain. DMA traffic is spread over the three DMA-capable queues
(sync/scalar/gpsimd); output stores ride gpsimd so they never block
the stacked-tile prefetches.
"""
import numpy as np

import concourse.bacc as bacc
import concourse.mybir as mybir
from concourse.bass_utils import run_bass_kernel_spmd
from concourse.tile import TileContext

P = 128
D = 512
TWO_D = 1024
NR = 2048
B = 8192
N_CORES = 8
BC = B // N_CORES      # 1024 batch rows per core
RT = BC // P           # 8 row tiles per core
KC = TWO_D // P        # 8 feature (contraction) chunks
NRC = NR // P          # 16 rel chunks
NLOC = NRC // N_CORES  # rel chunks per core for sharded R1
PEND = 7               # combine pipeline depth: the deferred combines
                       # pack the pipeline drain behind the last tile's
                       # serial DVE top-k chain

SA = 64.0              # stacked pre-scale (fp16-normal-range splits)
SB = 256.0             # rel pre-scale
ISCALE = 1.0 / (SA * SB)
NEG = -1.1e6           # sigmoid(NEG*ISCALE) ~ 8e-30: cancels exactly

F32 = mybir.dt.float32
F16 = mybir.dt.float16
AF = mybir.ActivationFunctionType

_CACHED = None


def _build():
    nc = bacc.Bacc("TRN2", target_bir_lowering=False, debug=True)
    # Host-marshalled operand layouts (see module docstring).
    a_hi_d = nc.declare_dram_parameter("A_hi", [RT * P, TWO_D], F16, isOutput=False)
    a_lo_d = nc.declare_dram_parameter("A_lo", [RT * P, TWO_D], F16, isOutput=False)
    rH = nc.declare_dram_parameter("relT_hi", [TWO_D, NR], F16, isOutput=False)
    rL = nc.declare_dram_parameter("relT_lo", [TWO_D, NR], F16, isOutput=False)
    wk = nc.declare_dram_parameter("W_k", [P, KC * D], F16, isOutput=False)
    bf = nc.declare_dram_parameter("b_fcs", [1, D], F16, isOutput=False)
    out = nc.declare_dram_parameter("out", [BC, D], F32, isOutput=True)

    with TileContext(nc) as tc:
        with (
            tc.tile_pool(name="consts", bufs=1) as consts,
            tc.tile_pool(name="persist", bufs=1) as persist,
            tc.tile_pool(name="psx", bufs=3, space="PSUM") as psx,
            tc.tile_pool(name="pso", bufs=2, space="PSUM") as pso,
            tc.tile_pool(name="work", bufs=2) as work,
            tc.tile_pool(name="comb", bufs=PEND + 1) as comb,
            # combines serialize on the PE, so M^T staging only needs a
            # short pipeline regardless of PEND
            tc.tile_pool(name="combt", bufs=3) as combt,
        ):
            ones1_f32 = consts.tile([1, P], F32)
            nc.vector.memset(ones1_f32, 1.0)
            ones1 = consts.tile([1, P], F16)
            nc.vector.tensor_copy(ones1, ones1_f32)

            # Tile-0 stacked splits lead the sync ring: the PE's first
            # gating matmul depends only on these + bh_0.
            a_tiles = {}

            def load_a(m):
                ah = work.tile([P, TWO_D], F16, tag="ah")
                nc.sync.dma_start(out=ah, in_=a_hi_d[m * P:(m + 1) * P, :])
                al = work.tile([P, TWO_D], F16, tag="al")
                nc.sync.dma_start(out=al, in_=a_lo_d[m * P:(m + 1) * P, :])
                a_tiles[m] = (ah, al)

            load_a(0)

            # Gating rel chunks round-robined across all three DMA rings
            # in need order (pair k feeds gating ~3us after pair k-1): a
            # ring completes its transfers roughly cumulative-bytes /
            # ring-bandwidth, so clustering the early chunks on one ring
            # starves the PE. The R1 operands (wt, b) trail on scalar —
            # R1 runs only after gating tile 0.
            bh_k = [None] * KC
            bl_k = [None] * KC
            rings = (nc.scalar, nc.gpsimd, nc.sync)
            for i in range(2 * KC):
                k, hi = divmod(i, 2)
                src = rH if hi == 0 else rL
                t = persist.tile([P, NR], F16,
                                 tag=f"{'bh' if hi == 0 else 'bl'}{k}")
                rings[i % 3].dma_start(out=t, in_=src[k * P:(k + 1) * P, :])
                (bh_k if hi == 0 else bl_k)[k] = t
            wt_sb = persist.tile([P, KC * D], F16)
            nc.scalar.dma_start(out=wt_sb, in_=wk[:])
            b_sb = consts.tile([1, D], F16)
            nc.scalar.dma_start(out=b_sb, in_=bf[:])

            # R1: rel-chunk c at cols [c*D, (c+1)*D), fp16 (value-grade).
            r1_sb = persist.tile([P, NRC * D], F16)

            def gating_phase(m):
                if m + 1 < RT:
                    load_a(m + 1)   # prefetch behind this tile's compute
                ah, al = a_tiles.pop(m)

                # Gating X*SA*SB via the 3-term fp16 split, fp32 PSUM
                # accumulation (24 matmuls per 512-col bank). k-outer so
                # each rel chunk pair (bh_k, bl_k) is fully consumed as it
                # streams in — the aggregate input-DMA rate, not any single
                # chunk, gates the head of the pipeline.
                xs = work.tile([P, NR], F32, tag="xs")
                xp0 = psx.tile([P, TWO_D], F32, tag="xph")
                xp1 = psx.tile([P, TWO_D], F32, tag="xph")
                xp = (xp0, xp1)
                for k in range(KC):
                    for t, (A, Bk) in enumerate(
                            ((ah, bh_k), (al, bh_k), (ah, bl_k))):
                        lhs = A[:, k * P:(k + 1) * P]
                        for hb in range(2):
                            for nb in range(2):
                                nc.tensor.matmul(
                                    xp[hb][:, nb * 512:(nb + 1) * 512],
                                    lhsT=lhs,
                                    rhs=Bk[k][:, (hb * 2 + nb) * 512:
                                              (hb * 2 + nb + 1) * 512],
                                    start=(t == 0 and k == 0),
                                    stop=(t == 2 and k == KC - 1),
                                )
                # PSUM->SBUF copies; on the last tile they sit on the
                # exposed drain chain, so split them across the scalar and
                # vector engines (distinct PSUM banks -> parallel access).
                for q in range(4):
                    src = xp[q // 2][:, (q % 2) * 512:(q % 2 + 1) * 512]
                    dst = xs[:, q * 512:(q + 1) * 512]
                    if m == RT - 1 and q % 2 == 1:
                        nc.vector.tensor_copy(dst, src)
                    else:
                        nc.scalar.activation(dst, src, AF.Copy)

                # Zap top-16 values (fp32 scan, selection-grade).
                m1 = work.tile([P, 8], F32, tag="m1")
                nc.vector.max(out=m1, in_=xs)
                xz = work.tile([P, NR], F32, tag="xz")
                nc.vector.match_replace(
                    out=xz, in_to_replace=m1, in_values=xs, imm_value=NEG)
                m2 = work.tile([P, 8], F32, tag="m2")
                nc.vector.max(out=m2, in_=xz)
                nc.vector.match_replace(
                    out=xz, in_to_replace=m2, in_values=xz, imm_value=NEG)

                # M = sigmoid(X) - sigmoid(X_zapped), fp16 (the
                # non-selected entries are identical fp16 values in both
                # sigmoids and cancel exactly); the 1/(SA*SB) rescale
                # rides the ACT input scale; denom via the activation
                # accumulators.
                s_all = work.tile([P, NR], F16, tag="s_all")
                acc_all = work.tile([P, 1], F32, tag="acc_all")
                nc.scalar.activation(
                    s_all, xs, AF.Sigmoid, scale=ISCALE, accum_out=acc_all)
                s_exc = work.tile([P, NR], F16, tag="s_exc")
                acc_exc = work.tile([P, 1], F32, tag="acc_exc")
                nc.scalar.activation(
                    s_exc, xz, AF.Sigmoid, scale=ISCALE, accum_out=acc_exc)
                mf = comb.tile([P, NR], F16, tag="mf")
                nc.vector.tensor_sub(mf, s_all, s_exc)
                den = work.tile([P, 1], F32, tag="den")
                nc.vector.tensor_sub(den, acc_all, acc_exc)
                rec = comb.tile([P, 1], F32, tag="rec")
                nc.vector.reciprocal(rec, den)
                return mf, rec

            def combine_phase(mm, mf, rec):
                # M^T via one xbar DMA: out[p, c, j] = in[j, c*P+p].
                mt = combt.tile([P, NRC * P], F16, tag="mt")
                nc.sync.dma_start_transpose(
                    mt[:].rearrange("p (c j) -> p c j", c=NRC), mf)
                op = pso.tile([P, D], F32, tag="pso")
                for c in range(NRC):
                    nc.tensor.matmul(
                        op,
                        lhsT=mt[:, c * P:(c + 1) * P],
                        rhs=r1_sb[:, c * D:(c + 1) * D],
                        start=(c == 0),
                        stop=(c == NRC - 1),
                    )
                ot = work.tile([P, D], F32, tag="ot")
                nc.scalar.activation(ot, op, AF.Copy, scale=rec)
                # Output stores on gpsimd: they trail the combine and must
                # not block the sync ring's stacked-tile prefetches.
                nc.gpsimd.dma_start(
                    out=out[mm * P:(mm + 1) * P, :], in_=ot)

            # Software pipeline: gating tile 0 first (PE starts as soon as
            # its operands stream in), then the full local R1 under it;
            # tile m's combine runs after tile m+PEND-1's gating so the PE
            # never waits on the serial DVE top-k chain.
            pending = [(0, *gating_phase(0))]

            # Full local R1 = relu(R @ W^T + b) on every core — no
            # collective (a cross-core AllGather measured ~90us of latency
            # and stalled the in-order PE queue). The lhsT operand is the
            # already-loaded scaled rel_hi chunks (256*R)^T; the 1/SB
            # rescale rides the ReLU's input scale, so the bias matmul
            # adds SB*b (pre-scaled host-side).
            for c in range(NRC):
                k0 = c * P
                pr = pso.tile([P, D], F32, tag="pso")
                for k in range(KC):
                    nc.tensor.matmul(
                        pr,
                        lhsT=bh_k[k][:, k0:k0 + P],
                        rhs=wt_sb[:, k * D:(k + 1) * D],
                        start=(k == 0),
                        stop=False,
                    )
                nc.tensor.matmul(
                    pr, lhsT=ones1, rhs=b_sb, start=False, stop=True)
                nc.scalar.activation(
                    r1_sb[:, c * D:(c + 1) * D], pr, AF.Relu,
                    scale=1.0 / SB)

            for m in range(1, RT):
                pending.append((m, *gating_phase(m)))
                if len(pending) >= PEND:
                    combine_phase(*pending.pop(0))
            while pending:
                combine_phase(*pending.pop(0))

    nc.finalize()
    return nc


def _get_nc():
    global _CACHED
    if _CACHED is None:
        _CACHED = _build()
    return _CACHED


def _split16(x):
    hi = x.astype(np.float16)
    lo = (x - hi.astype(np.float32)).astype(np.float16)
    return hi, lo


def _chunk_part(x):
    """[TWO_D, N] -> [P, KC*N]: feature-chunk k at cols [k*N, (k+1)*N)."""
    n = x.shape[1]
    return np.ascontiguousarray(
        x.reshape(KC, P, n).transpose(1, 0, 2).reshape(P, KC * n))


def _make_in_maps(e1, e2, rel_emb, W_fcs, b_fcs):
    e1 = np.asarray(e1, dtype=np.float32)
    e2 = np.asarray(e2, dtype=np.float32)
    rel_emb = np.asarray(rel_emb, dtype=np.float32)
    W_fcs = np.asarray(W_fcs, dtype=np.float32)
    b_fcs = np.asarray(b_fcs, dtype=np.float32).reshape(1, D)

    stacked = np.concatenate([e1, e2], axis=1) * SA   # [B, 2D]
    a_hi, a_lo = _split16(stacked)
    relT = np.ascontiguousarray((rel_emb * SB).T)     # [2D, NR]
    r_hi, r_lo = _split16(relT)

    # A tiles: [RT*P, TWO_D] with A[m*P+p, k*P+j] = stacked[m*P+j, k*P+p]
    def a_tiles(a):
        return np.ascontiguousarray(
            a.reshape(RT, P, KC, P).transpose(0, 3, 2, 1).reshape(RT * P, TWO_D))

    wkm = _chunk_part(np.ascontiguousarray(W_fcs.T)).astype(np.float16)
    return [
        {
            "A_hi": a_tiles(a_hi[c * BC:(c + 1) * BC]),
            "A_lo": a_tiles(a_lo[c * BC:(c + 1) * BC]),
            "relT_hi": r_hi,
            "relT_lo": r_lo,
            "W_k": wkm,
            # the R1 bias matmul adds SB*b (rescaled away inside the ReLU)
            "b_fcs": (b_fcs * SB).astype(np.float16),
        }
        for c in range(N_CORES)
    ]


def kernel(e1, e2, rel_emb, W_fcs, b_fcs, **_ignored):
    nc = _get_nc()
    in_maps = _make_in_maps(e1, e2, rel_emb, W_fcs, b_fcs)
    res = run_bass_kernel_spmd(nc, in_maps, list(range(N_CORES)))
    return np.concatenate(
        [res.results[c]["out"] for c in range(N_CORES)], axis=0)
